# revision 8
# baseline (speedup 1.0000x reference)
import sys
sys.path.insert(0, "/opt/trn_rl_repo")
import numpy as np
import concourse.bass as bass
import concourse.mybir as mybir
import concourse.tile as tile
from concourse import bacc
from concourse.bass_utils import run_bass_kernel_spmd
from concourse.masks import make_identity

F32 = mybir.dt.float32
BF16 = mybir.dt.bfloat16
AF = mybir.ActivationFunctionType
OP = mybir.AluOpType
AX = mybir.AxisListType

S = 2048          # sequence length
H = 4096          # hidden dim
DH = 128          # head dim
NQ = 4            # q heads per core (32 / 8)
NT = S // 128     # 16 q tiles of 128
NCORES = 8
SCALE = 1.0 / np.sqrt(128.0)
NEG = -1.0e33

_CACHED = {}


def _phase_b(nc, tc, hidT_d, wqkvT_d, qT, kT, vT, cosb, sinb):
    # kt-major: for each 512-col seq chunk, stream hid k-tiles and issue all
    # 6 output-row matmuls per k-tile so PE work starts as soon as the first
    # (w, hid) tile pair lands instead of after a full chunk of DMA.
    with tc.tile_pool(name="wqp", bufs=1) as wq_p, \
         tc.tile_pool(name="hid", bufs=2) as hid_p, \
         tc.tile_pool(name="bps", bufs=1, space="PSUM") as bps, \
         tc.tile_pool(name="stg", bufs=2) as stg_p, \
         tc.tile_pool(name="rt", bufs=2) as rt_p:
        wqs = []
        for kt in range(32):
            w = wq_p.tile([128, 768], BF16, tag=f"wq{kt}")
            nc.sync.dma_start(w, wqkvT_d[kt * 128:(kt + 1) * 128, :])
            wqs.append(w)

        warmed = False
        for sc in range(4):
            ssl = slice(sc * 512, (sc + 1) * 512)
            pss = [bps.tile([128, 512], F32, tag=f"bacc{m}", name=f"bacc{m}")
                   for m in range(6)]
            for kt in range(32):
                ht = hid_p.tile([128, 512], BF16, tag=f"h{kt % 8}")
                eng = nc.gpsimd if kt % 2 == 0 else nc.scalar
                eng.dma_start(ht, hidT_d[kt * 128:(kt + 1) * 128, ssl])
                for m in range(6):
                    nc.tensor.matmul(
                        pss[m], wqs[kt][:, m * 128:(m + 1) * 128], ht,
                        start=(kt == 0), stop=(kt == 31))
                if not warmed and kt == 2:
                    # pre-warm exp table after first tiles issue so the
                    # ~2.7us table load doesn't delay the first hid tiles
                    warm = rt_p.tile([128, 1], BF16, tag="warm")
                    nc.scalar.activation(warm, cosb[:, 0:1], AF.Exp)
                    warmed = True
            # drain: rope math reads PSUM directly (cross-partition SBUF
            # reads are rejected by the verifier); vT copy goes to ACT so
            # DVE only handles the rope chains.
            nc.scalar.copy(vT[:, ssl], pss[5])
            for m in range(5):
                dst = qT[:, m, ssl] if m < 4 else kT[:, ssl]
                ps = pss[m]
                t1 = rt_p.tile([128, 512], F32, tag="t1")
                t2 = rt_p.tile([128, 512], F32, tag="t2")
                nc.vector.tensor_mul(t1, ps, cosb[:, ssl])
                nc.vector.tensor_mul(t2[0:64], ps[64:128], sinb[0:64, ssl])
                nc.vector.tensor_mul(t2[64:128], ps[0:64], sinb[64:128, ssl])
                nc.vector.tensor_add(dst, t1, t2)


def _phase_c(nc, tc, qT, kT, vT, vnat, attnT, identb, dmask4, ones_col,
             ones_row):
    # Transposed-score attention: st^T[k, (h,q)] = k_tile^T @ q_4heads in one
    # N=512 matmul; exp(st^T) is directly the PV rhs (no PE transposes).
    # Denominator via ones-column matmul chain; normalization folded into
    # the PV drain with a rank-1 broadcast matmul of the reciprocals.
    with tc.tile_pool(name="tps", bufs=2, space="PSUM") as tr_p:
        for g in range(2):
            tp = tr_p.tile([128, 8, 128], BF16, tag="tp")
            for i in range(8):
                st8 = 8 * g + i
                nc.tensor.transpose(
                    tp[:, i, :], vT[:, st8 * 128:(st8 + 1) * 128], identb)
            nc.vector.tensor_copy(vnat[:, 8 * g:8 * g + 8, :], tp)

    with tc.tile_pool(name="stp", bufs=3, space="PSUM") as st_p, \
         tc.tile_pool(name="pvp", bufs=2, space="PSUM") as pv_p, \
         tc.tile_pool(name="dnp", bufs=2, space="PSUM") as dn_p, \
         tc.tile_pool(name="ptp", bufs=3) as pt_p, \
         tc.tile_pool(name="rcp", bufs=2) as rc_p:
        pv_cur = {}
        dn_cur = {}

        def make_pvden(t, kb, pt):
            def th():
                if kb == 0:
                    pv_cur[t] = pv_p.tile([128, NQ, 128], F32, tag="pv", name="pv")
                    dn_cur[t] = dn_p.tile([1, NQ, 128], F32, tag="dn", name="dn")
                nc.tensor.matmul(pv_cur[t], vnat[:, kb, :], pt,
                                 start=(kb == 0), stop=(kb == t))
                nc.tensor.matmul(dn_cur[t], ones_col[:, 0:1], pt,
                                 start=(kb == 0), stop=(kb == t))
            return th

        def make_fin(t):
            def th():
                rc = rc_p.tile([1, NQ, 128], BF16, tag="rc")
                with nc.allow_low_precision(
                        reason="bf16 recip matches baseline prob precision"):
                    nc.vector.reciprocal(rc, dn_cur[t])
                rb = rc_p.tile([128, NQ, 128], BF16, tag="rb")
                nc.gpsimd.partition_broadcast(rb, rc)
                nc.vector.tensor_mul(
                    attnT[:, :, t * 128:(t + 1) * 128], pv_cur[t], rb)
            return th

        steps = [(t, kb) for t in range(NT) for kb in range(t + 1)]
        deferred = {}
        for i, (t, kb) in enumerate(steps):
            for th in deferred.pop(i, []):
                th()
            st = st_p.tile([128, NQ, 128], F32, tag="st")
            nc.tensor.matmul(st, kT[:, kb * 128:(kb + 1) * 128],
                             qT[:, :, t * 128:(t + 1) * 128],
                             start=True, stop=True)
            if kb == t:
                nc.vector.tensor_add(st, st, dmask4)
            pt = pt_p.tile([128, NQ, 128], BF16, tag="pt")
            nc.scalar.activation(pt, st, AF.Exp, scale=SCALE)
            deferred.setdefault(i + 1, []).append(make_pvden(t, kb, pt))
            if kb == t:
                deferred.setdefault(i + 3, []).append(make_fin(t))
        for i in sorted(deferred):
            for th in deferred[i]:
                th()


def _phase_d(nc, tc, wogs, attnT, out_d):
    with tc.tile_pool(name="dps", bufs=4, space="PSUM") as dps, \
         tc.tile_pool(name="ob", bufs=4) as ob_p:
        for m in range(32):
            mg, mo = divmod(m, 4)
            for scc in range(4):
                po = dps.tile([128, 512], F32, tag="po")
                for a in range(NQ):
                    nc.tensor.matmul(
                        po, wogs[mg][:, a, mo * 128:(mo + 1) * 128],
                        attnT[:, a, scc * 512:(scc + 1) * 512],
                        start=(a == 0), stop=(a == NQ - 1))
                ob = ob_p.tile([128, 512], BF16, tag="ob")
                if (m * 4 + scc) % 2 == 0:
                    nc.vector.tensor_copy(ob, po)
                else:
                    nc.scalar.copy(ob, po)
                nc.sync.dma_start(
                    out_d[m * 128:(m + 1) * 128, scc * 512:(scc + 1) * 512],
                    ob)


def _build_nc():
    nc = bacc.Bacc(None, target_bir_lowering=False, debug=False)
    # Inputs host-pre-transposed/cast so no PE transposes are needed:
    #   hidt  = hidden[0].T            [H, S]
    #   wqkvt = [Wq_c; Wk_c; Wv_c].T   [H, 768]   (cols 0:512 q, 512:640 k, 640:768 v)
    #   wot   = Wo[:, c*512:...].T     [512, H]
    #   cos/sin [d=128, S], sin sign-folded for rotate_half
    hidT_d = nc.dram_tensor("hidt", [H, S], BF16, kind="ExternalInput")
    wqkvT_d = nc.dram_tensor("wqkvt", [H, 768], BF16, kind="ExternalInput")
    woT_d = nc.dram_tensor("wot", [NQ * DH, H], BF16, kind="ExternalInput")
    cos_d = nc.dram_tensor("cos", [DH, S], BF16, kind="ExternalInput")
    sin_d = nc.dram_tensor("sin", [DH, S], BF16, kind="ExternalInput")
    out_d = nc.dram_tensor("outt", [H, S], BF16, kind="ExternalOutput")

    with tile.TileContext(nc) as tc:
        with tc.tile_pool(name="perm", bufs=1) as perm:
            identf = perm.tile([128, 128], F32, tag="identf")
            make_identity(nc, identf)
            identb = perm.tile([128, 128], BF16, tag="identb")
            nc.vector.tensor_copy(identb, identf)
            # additive causal mask for the diagonal 128x128 block in the
            # TRANSPOSED [k, q] layout: 0 where q >= k, NEG where q < k
            # (k = partition, q = free); replicated for the 4 heads.
            zeros = perm.tile([128, 128], F32, tag="zeros")
            nc.gpsimd.memset(zeros, 0.0)
            dmask4 = perm.tile([128, NQ, 128], F32, tag="dmask4")
            for h in range(NQ):
                nc.gpsimd.affine_select(
                    out=dmask4[:, h, :], in_=zeros, pattern=[[1, 128]],
                    compare_op=OP.is_ge, fill=NEG,
                    base=0, channel_multiplier=-1)
            ones_col = perm.tile([128, 1], BF16, tag="ones_col")
            nc.gpsimd.memset(ones_col, 1.0)
            ones_row = perm.tile([1, 128], BF16, tag="ones_row")
            nc.gpsimd.memset(ones_row, 1.0)

            # persistent strips (bf16)
            qT = perm.tile([128, NQ, S], BF16, tag="qT")
            kT = perm.tile([128, S], BF16, tag="kT")
            vT = perm.tile([128, S], BF16, tag="vT")
            vnat = perm.tile([128, NT, 128], BF16, tag="vnat")
            attnT = perm.tile([128, NQ, S], BF16, tag="attnT")
            cosb = perm.tile([128, S], BF16, tag="cosb")
            sinb = perm.tile([128, S], BF16, tag="sinb")
            nc.sync.dma_start(cosb, cos_d[:, :])
            nc.sync.dma_start(sinb, sin_d[:, :])

            _phase_b(nc, tc, hidT_d, wqkvT_d, qT, kT, vT, cosb, sinb)

            # o_proj weights: load early on the (now idle) sync queue
            with tc.tile_pool(name="wo", bufs=1) as wo_p:
                wogs = []
                for mg in range(8):
                    wg = wo_p.tile([128, NQ, 512], BF16, tag=f"wo{mg}")
                    for a in range(NQ):
                        nc.sync.dma_start(
                            wg[:, a, :],
                            woT_d[a * 128:(a + 1) * 128, mg * 512:(mg + 1) * 512])
                    wogs.append(wg)

                _phase_c(nc, tc, qT, kT, vT, vnat, attnT, identb, dmask4,
                         ones_col, ones_row)
                _phase_d(nc, tc, wogs, attnT, out_d)
    nc.compile()
    return nc


def _prep_inputs(hidden_states, position_ids, Wq, Wk, Wv, Wo):
    bf16 = np.dtype(mybir.dt.np(BF16))
    hs = np.asarray(hidden_states, dtype=np.float32)
    hidT = np.ascontiguousarray(hs[0].T).astype(bf16)

    pos = np.asarray(position_ids).reshape(-1).astype(np.float64)
    invf = 1.0 / (10000.0 ** (np.arange(0, 128, 2, dtype=np.float64) / 128.0))
    ang = invf[:, None] * pos[None, :]
    cos_t = np.concatenate([np.cos(ang), np.cos(ang)], axis=0).astype(bf16)
    sin_t = np.concatenate([-np.sin(ang), np.sin(ang)], axis=0).astype(bf16)

    Wq = np.asarray(Wq, dtype=np.float32)
    Wk = np.asarray(Wk, dtype=np.float32)
    Wv = np.asarray(Wv, dtype=np.float32)
    Wo = np.asarray(Wo, dtype=np.float32)
    in_maps = []
    for c in range(NCORES):
        wqkv = np.concatenate([
            Wq[c * 512:(c + 1) * 512],
            Wk[c * 128:(c + 1) * 128],
            Wv[c * 128:(c + 1) * 128]], axis=0)          # [768, H]
        wqkvT = np.ascontiguousarray(wqkv.T).astype(bf16)  # [H, 768]
        woT = np.ascontiguousarray(Wo[:, c * 512:(c + 1) * 512].T).astype(bf16)
        in_maps.append({"hidt": hidT, "wqkvt": wqkvT, "wot": woT,
                        "cos": cos_t, "sin": sin_t})
    return in_maps


def kernel(hidden_states, position_ids, Wq, Wk, Wv, Wo, **extra):
    hs = np.asarray(hidden_states)
    B = hs.shape[0]
    assert B == 1 and hs.shape[1] == S and hs.shape[2] == H

    if "nc" not in _CACHED:
        _CACHED["nc"] = _build_nc()
    nc = _CACHED["nc"]

    in_maps = _prep_inputs(hidden_states, position_ids, Wq, Wk, Wv, Wo)
    res = run_bass_kernel_spmd(nc, in_maps, core_ids=list(range(NCORES)))
    out = np.zeros((H, S), dtype=np.float32)
    for c in range(NCORES):
        out += np.asarray(res.results[c]["outt"]).astype(np.float32)
    return np.ascontiguousarray(out.T).reshape(1, S, H)


# revision 10
# speedup vs baseline: 1.0855x; 1.0855x over previous
import sys
sys.path.insert(0, "/opt/trn_rl_repo")
import numpy as np
import concourse.bass as bass
import concourse.mybir as mybir
import concourse.tile as tile
from concourse import bacc
from concourse.bass_utils import run_bass_kernel_spmd
from concourse.masks import make_identity

F32 = mybir.dt.float32
BF16 = mybir.dt.bfloat16
AF = mybir.ActivationFunctionType
OP = mybir.AluOpType
AX = mybir.AxisListType

S = 2048          # sequence length
H = 4096          # hidden dim
DH = 128          # head dim
NQ = 4            # q heads per core (32 / 8)
NT = S // 128     # 16 q tiles of 128
NCORES = 8
SCALE = 1.0 / np.sqrt(128.0)
NEG = -1.0e33

_CACHED = {}


def _phase_b(nc, tc, hidT_d, wqkvT_d, cos_d, sin_d, qT, kT, vT, cosb, sinb, identf):
    # kt-major: for each 512-col seq chunk, stream hid k-tiles and issue all
    # 6 output-row matmuls per k-tile so PE work starts as soon as the first
    # (w, hid) tile pair lands instead of after a full chunk of DMA.
    with tc.tile_pool(name="wqp", bufs=1) as wq_p, \
         tc.tile_pool(name="hid", bufs=2) as hid_p, \
         tc.tile_pool(name="bps", bufs=1, space="PSUM") as bps, \
         tc.tile_pool(name="stg", bufs=2) as stg_p, \
         tc.tile_pool(name="rt", bufs=2) as rt_p:
        wqs = []
        for kt in range(32):
            w = wq_p.tile([128, 768], BF16, tag=f"wq{kt}")
            nc.sync.dma_start(w, wqkvT_d[kt * 128:(kt + 1) * 128, :])
            wqs.append(w)

        nc.sync.dma_start(cosb, cos_d[:, :])
        nc.sync.dma_start(sinb, sin_d[:, :])
        warmed = False
        for sc in range(4):
            ssl = slice(sc * 512, (sc + 1) * 512)
            pss = [bps.tile([128, 512], F32, tag=f"bacc{m}", name=f"bacc{m}")
                   for m in range(6)]
            for kt in range(32):
                ht = hid_p.tile([128, 512], BF16, tag=f"h{kt % 8}")
                eng = nc.gpsimd if kt % 2 == 0 else nc.scalar
                eng.dma_start(ht, hidT_d[kt * 128:(kt + 1) * 128, ssl])
                for m in range(6):
                    nc.tensor.matmul(
                        pss[m], wqs[kt][:, m * 128:(m + 1) * 128], ht,
                        start=(kt == 0), stop=(kt == 31))
                if not warmed and kt == 2:
                    # pre-warm exp table after first tiles issue so the
                    # ~2.7us table load doesn't delay the first hid tiles
                    warm = rt_p.tile([128, 1], BF16, tag="warm")
                    nc.scalar.activation(warm, identf[:, 0:1], AF.Exp)
                    warmed = True
            # drain: rope math reads PSUM directly (cross-partition SBUF
            # reads are rejected by the verifier); vT copy goes to ACT so
            # DVE only handles the rope chains.
            nc.scalar.copy(vT[:, ssl], pss[5])
            for m in range(5):
                dst = qT[:, m, ssl] if m < 4 else kT[:, ssl]
                ps = pss[m]
                t1 = rt_p.tile([128, 512], F32, tag="t1")
                t2 = rt_p.tile([128, 512], F32, tag="t2")
                nc.vector.tensor_mul(t1, ps, cosb[:, ssl])
                nc.vector.tensor_mul(t2[0:64], ps[64:128], sinb[0:64, ssl])
                nc.vector.tensor_mul(t2[64:128], ps[0:64], sinb[64:128, ssl])
                nc.vector.tensor_add(dst, t1, t2)


def _phase_c(nc, tc, qT, kT, vT, vnat, attnT, identb, dmask4, ones_sq):
    # Transposed-score attention: st^T[k, (h,q)] = k_tile^T @ q_4heads in one
    # N=512 matmul; exp(st^T) is directly the PV rhs (no PE transposes).
    # Denominator via ones-column matmul chain; normalization folded into
    # the PV drain with a rank-1 broadcast matmul of the reciprocals.
    with tc.tile_pool(name="tps", bufs=2, space="PSUM") as tr_p:
        for g in range(2):
            tp = tr_p.tile([128, 8, 128], BF16, tag="tp")
            for i in range(8):
                st8 = 8 * g + i
                nc.tensor.transpose(
                    tp[:, i, :], vT[:, st8 * 128:(st8 + 1) * 128], identb)
            nc.vector.tensor_copy(vnat[:, 8 * g:8 * g + 8, :], tp)

    with tc.tile_pool(name="stp", bufs=3, space="PSUM") as st_p, \
         tc.tile_pool(name="pvp", bufs=2, space="PSUM") as pv_p, \
         tc.tile_pool(name="dnp", bufs=2, space="PSUM") as dn_p, \
         tc.tile_pool(name="ptp", bufs=3) as pt_p, \
         tc.tile_pool(name="rcp", bufs=2) as rc_p:
        pv_cur = {}
        dn_cur = {}

        def make_pvden(t, kb, pt):
            def th():
                if kb == 0:
                    pv_cur[t] = pv_p.tile([128, NQ, 128], F32, tag="pv", name="pv")
                    dn_cur[t] = dn_p.tile([128, NQ, 128], F32, tag="dn", name="dn")
                nc.tensor.matmul(pv_cur[t], vnat[:, kb, :], pt,
                                 start=(kb == 0), stop=(kb == t))
                nc.tensor.matmul(dn_cur[t], ones_sq, pt,
                                 start=(kb == 0), stop=(kb == t))
            return th

        def make_fin(t):
            def th():
                rc = rc_p.tile([128, NQ, 128], BF16, tag="rc")
                with nc.allow_low_precision(
                        reason="bf16 recip matches baseline prob precision"):
                    nc.vector.reciprocal(rc, dn_cur[t])
                nc.vector.tensor_mul(
                    attnT[:, :, t * 128:(t + 1) * 128], pv_cur[t], rc)
            return th

        steps = [(t, kb) for t in range(NT) for kb in range(t + 1)]
        deferred = {}
        for i, (t, kb) in enumerate(steps):
            for th in deferred.pop(i, []):
                th()
            st = st_p.tile([128, NQ, 128], F32, tag="st")
            nc.tensor.matmul(st, kT[:, kb * 128:(kb + 1) * 128],
                             qT[:, :, t * 128:(t + 1) * 128],
                             start=True, stop=True)
            if kb == t:
                nc.vector.tensor_add(st, st, dmask4)
            pt = pt_p.tile([128, NQ, 128], BF16, tag="pt")
            nc.scalar.activation(pt, st, AF.Exp, scale=SCALE)
            deferred.setdefault(i + 1, []).append(make_pvden(t, kb, pt))
            if kb == t:
                deferred.setdefault(i + 3, []).append(make_fin(t))
        for i in sorted(deferred):
            for th in deferred[i]:
                th()


def _phase_d(nc, tc, wogs, attnT, out_d):
    with tc.tile_pool(name="dps", bufs=4, space="PSUM") as dps, \
         tc.tile_pool(name="ob", bufs=4) as ob_p:
        for m in range(32):
            mg, mo = divmod(m, 4)
            for scc in range(4):
                po = dps.tile([128, 512], F32, tag="po")
                for a in range(NQ):
                    nc.tensor.matmul(
                        po, wogs[mg][:, a, mo * 128:(mo + 1) * 128],
                        attnT[:, a, scc * 512:(scc + 1) * 512],
                        start=(a == 0), stop=(a == NQ - 1))
                ob = ob_p.tile([128, 512], BF16, tag="ob")
                if (m * 4 + scc) % 2 == 0:
                    nc.vector.tensor_copy(ob, po)
                else:
                    nc.scalar.copy(ob, po)
                nc.sync.dma_start(
                    out_d[m * 128:(m + 1) * 128, scc * 512:(scc + 1) * 512],
                    ob)


def _build_nc():
    nc = bacc.Bacc(None, target_bir_lowering=False, debug=False)
    # Inputs host-pre-transposed/cast so no PE transposes are needed:
    #   hidt  = hidden[0].T            [H, S]
    #   wqkvt = [Wq_c; Wk_c; Wv_c].T   [H, 768]   (cols 0:512 q, 512:640 k, 640:768 v)
    #   wot   = Wo[:, c*512:...].T     [512, H]
    #   cos/sin [d=128, S], sin sign-folded for rotate_half
    hidT_d = nc.dram_tensor("hidt", [H, S], BF16, kind="ExternalInput")
    wqkvT_d = nc.dram_tensor("wqkvt", [H, 768], BF16, kind="ExternalInput")
    woT_d = nc.dram_tensor("wot", [NQ * DH, H], BF16, kind="ExternalInput")
    cos_d = nc.dram_tensor("cos", [DH, S], BF16, kind="ExternalInput")
    sin_d = nc.dram_tensor("sin", [DH, S], BF16, kind="ExternalInput")
    out_d = nc.dram_tensor("outt", [H, S], BF16, kind="ExternalOutput")

    with tile.TileContext(nc) as tc:
        with tc.tile_pool(name="perm", bufs=1) as perm:
            identf = perm.tile([128, 128], F32, tag="identf")
            make_identity(nc, identf)
            identb = perm.tile([128, 128], BF16, tag="identb")
            nc.vector.tensor_copy(identb, identf)
            # additive causal mask for the diagonal 128x128 block in the
            # TRANSPOSED [k, q] layout: 0 where q >= k, NEG where q < k
            # (k = partition, q = free); replicated for the 4 heads.
            zeros = perm.tile([128, 128], F32, tag="zeros")
            nc.gpsimd.memset(zeros, 0.0)
            dmask4 = perm.tile([128, NQ, 128], F32, tag="dmask4")
            for h in range(NQ):
                nc.gpsimd.affine_select(
                    out=dmask4[:, h, :], in_=zeros, pattern=[[1, 128]],
                    compare_op=OP.is_ge, fill=NEG,
                    base=0, channel_multiplier=-1)
            ones_sq = perm.tile([128, 128], BF16, tag="ones_sq")
            nc.gpsimd.memset(ones_sq, 1.0)

            # persistent strips (bf16)
            qT = perm.tile([128, NQ, S], BF16, tag="qT")
            kT = perm.tile([128, S], BF16, tag="kT")
            vT = perm.tile([128, S], BF16, tag="vT")
            vnat = perm.tile([128, NT, 128], BF16, tag="vnat")
            attnT = perm.tile([128, NQ, S], BF16, tag="attnT")
            cosb = perm.tile([128, S], BF16, tag="cosb")
            sinb = perm.tile([128, S], BF16, tag="sinb")
            _phase_b(nc, tc, hidT_d, wqkvT_d, cos_d, sin_d, qT, kT, vT, cosb, sinb, identf)

            # o_proj weights: load early on the (now idle) sync queue
            with tc.tile_pool(name="wo", bufs=1) as wo_p:
                wogs = []
                for mg in range(8):
                    wg = wo_p.tile([128, NQ, 512], BF16, tag=f"wo{mg}")
                    for a in range(NQ):
                        nc.sync.dma_start(
                            wg[:, a, :],
                            woT_d[a * 128:(a + 1) * 128, mg * 512:(mg + 1) * 512])
                    wogs.append(wg)

                _phase_c(nc, tc, qT, kT, vT, vnat, attnT, identb, dmask4,
                         ones_sq)
                _phase_d(nc, tc, wogs, attnT, out_d)
    nc.compile()
    return nc


def _prep_inputs(hidden_states, position_ids, Wq, Wk, Wv, Wo):
    bf16 = np.dtype(mybir.dt.np(BF16))
    hs = np.asarray(hidden_states, dtype=np.float32)
    hidT = np.ascontiguousarray(hs[0].T).astype(bf16)

    pos = np.asarray(position_ids).reshape(-1).astype(np.float64)
    invf = 1.0 / (10000.0 ** (np.arange(0, 128, 2, dtype=np.float64) / 128.0))
    ang = invf[:, None] * pos[None, :]
    cos_t = np.concatenate([np.cos(ang), np.cos(ang)], axis=0).astype(bf16)
    sin_t = np.concatenate([-np.sin(ang), np.sin(ang)], axis=0).astype(bf16)

    Wq = np.asarray(Wq, dtype=np.float32)
    Wk = np.asarray(Wk, dtype=np.float32)
    Wv = np.asarray(Wv, dtype=np.float32)
    Wo = np.asarray(Wo, dtype=np.float32)
    in_maps = []
    for c in range(NCORES):
        wqkv = np.concatenate([
            Wq[c * 512:(c + 1) * 512],
            Wk[c * 128:(c + 1) * 128],
            Wv[c * 128:(c + 1) * 128]], axis=0)          # [768, H]
        wqkvT = np.ascontiguousarray(wqkv.T).astype(bf16)  # [H, 768]
        woT = np.ascontiguousarray(Wo[:, c * 512:(c + 1) * 512].T).astype(bf16)
        in_maps.append({"hidt": hidT, "wqkvt": wqkvT, "wot": woT,
                        "cos": cos_t, "sin": sin_t})
    return in_maps


def kernel(hidden_states, position_ids, Wq, Wk, Wv, Wo, **extra):
    hs = np.asarray(hidden_states)
    B = hs.shape[0]
    assert B == 1 and hs.shape[1] == S and hs.shape[2] == H

    if "nc" not in _CACHED:
        _CACHED["nc"] = _build_nc()
    nc = _CACHED["nc"]

    in_maps = _prep_inputs(hidden_states, position_ids, Wq, Wk, Wv, Wo)
    res = run_bass_kernel_spmd(nc, in_maps, core_ids=list(range(NCORES)))
    out = np.zeros((H, S), dtype=np.float32)
    for c in range(NCORES):
        out += np.asarray(res.results[c]["outt"]).astype(np.float32)
    return np.ascontiguousarray(out.T).reshape(1, S, H)
